# revision 69
# baseline (speedup 1.0000x reference)
"""ConvLSTM classifier kernel for Trainium2 (8 NeuronCores, data-parallel). v2

Math (per core, batch shard BL=2048):
  for t in 0..T-1:
    gates = conv1d(x_t, w_ih) + conv1d(h, w_hh) + bias     # (BL, 20, 64), 'SAME' K=5
    i,f,g,o = split(gates); i,f,o = sigmoid; g = tanh
    c = f*c + i*g ; h = o*tanh(c)
  logit = h . fc_w + fc_b ; p = sigmoid(logit)
  out = 1 - prod_c(1-p_c) * (1-sigmoid(baseline))

Design (v2.5, ~426-449us/core measured, run-to-run thermal variance ~3%;
ACT engine ~85% active = the algorithm's sigmoid/tanh stream is the wall):
  - J-MAJOR tap order (tap = j*5+c) and lam-major gate columns
    (col = G*40 + lam*5 + ch): halo copies are packed 10-element runs and
    the h-scatter is one contiguous 40-run per window block (the (c,j)
    order forced 2-element bursts that cost ~520ns per halo copy).
  - v = i*tanh(g) = (sigmoid(2g)*2-1)*i in ONE AFFINE_MUL_REDUCE custom
    DVE op (exact; replaces the tensor_scalar + tensor_tensor pair).
  - tanh(c) is FUSED into the h-scatter: h = sigma_o * tanh3(c) in ONE
    custom DVE uop (TANHMUL_ANT, 7 ALU blocks: clamp+cubic+multiply,
    strided sigma_o as in0, contiguous cn as the required 1-D in1).
    No separate tanh instruction, no tquad intermediate, shorter
    cn -> h -> transpose chain.  End-to-end rel err 1.2e-3.
  - Quad blocks (tanh + h-scatter + halos) are emitted deferred by one
    pair: the quad tanh's cn input is then always ready when the engine
    reaches it, so it never parks in the 4-deep wait queue blocking
    ready sigmoids behind it (measured ~1.8us/step ACT stall).
  - Halos: left is a DVE copy, right a gpsimd copy (parallel engines).
    A gpsimd op must NOT read the ifog pool tile (its WAR release then
    makes later ACT sigmoids wait on the slow gpsimd queue; +98us).
  - v and fc live in QUAD-sized tiles written in pair halves, so the
    cn = fc + v add is one 1280-elem TT per quad (halves the dispatch
    count of the DVE bottleneck's cheapest-per-element op).
  - Final sigmoid/product/combine on HOST; device ships raw logits.
  - Rejected by measurement: fused x|h matmuls + per-gate ACT slices
    (794us: chip power-throttle caps utilization when 5 engines run hot,
    and small strided-PSUM ACT instructions carry ~260ns fixed cost);
    gpsimd h-multiply (throttle); 2-step x prefetch and split x DMA
    (DMA contention with the transposes); halos both on gpsimd (the
    h->halo->transpose chain gates the next step's matmuls, +17us);
    halos recomputed as gpsimd o*tanh TTs (+98us: they read the ifog
    pool tile inside the deferred block, so the pool's WAR dep makes
    later ACT sigmoids wait on the slow gpsimd queue); ifog bufs 6->8
    (neutral-to-worse); halo-left as a small SBUF->SBUF DMA on the sync
    queue (+290us!! tiny DMAs wreck the HWDGE ring that also carries
    the x stream and h transposes, despite ample chain slack).

Original v2 design notes:
  - x im2col is pre-transposed ON HOST into pair-block layout; one
    contiguous [128, 8192] HWDGE DMA per timestep (issued at the top of
    each t so the SP ring never head-of-line blocks; t=0 split per pair
    so pair 0's matmuls start early).
  - Only h goes through the on-device DMA-xbar transpose, at 64-tap
    pitch: [128 b, 1024] -> [128, 8 blk, 128 b] per bg-PAIR.
  - Beta-interleaved pair blocks: stationary tile block w holds window
    w's taps for BOTH bgs of a pair (bg0 taps at partitions 0-63, bg1
    at 64-127); rhs weights are block-diagonal [64x160 | 64x160] so one
    N=320 matmul yields window w's gates for both bgs.  Per pair: 8
    window-MM pairs (x start / h stop, PSUM-accumulated), 4-bank PSUM
    slot per half, 2 slots rotating.
  - Bias enters via a constant-1.0 row (tap 60) in the HOST x data;
    g-gate weights doubled so a single Sigmoid pass covers all four
    gates (tanh(g) = 2*sigmoid(2g)-1).
  - ACT (the bottleneck engine, ~84% busy): one 1280-elem Sigmoid per
    4-window half + one 1280-elem Tanh per QUAD (4 bgs) + final FC.
  - DVE cell update at bg-PAIR granularity (640-elem ops): v=i*g,
    u=2v-i (STT), fc=f*c, cn=fc+u, h-scatter TT; the two beta-merged
    halo copies split GpSimd/DVE so they run in parallel.
  - Final FC emitted inside the t=15 scatter block per pair (pair-wide
    sigmoid/product), overlapping the remaining pairs' last-step work.
"""

import numpy as np

import concourse.bass as bass
import concourse.bacc as bacc
import concourse.tile as tile
import concourse.mybir as mybir
from concourse import bass_utils

dt = mybir.dt
ALU = mybir.AluOpType
ACT = mybir.ActivationFunctionType

TIME = 16
BATCH = 16384
C = 5
L = 64
NCORES = 8
BL = BATCH // NCORES          # 2048 per core
NBG = BL // 128               # 16 batch groups
NPAIR = NBG // 2              # 8 bg pairs
NW = 8                        # l-windows per batch row (l_seg = 8)
WJ = 12                       # taps per (window, channel): 8 + 4 halo
NK = 4                        # window-pairs per bg
BIAS_TAP = 60                 # constant-1.0 row inside each 64-tap half

# clamped-cubic tanh fit for tanh(c): t ~ u*(TA + TB*u^2), u = clip(c,-TL,TL)
# (|c| measured <= 1.53; end-to-end rel err vs oracle 1.2e-3, tol 2e-2)
TL = 2.0
TA = 0.852982
TB = -0.097015

_TANHMUL = None


def _get_tanhmul():
    """Custom DVE op out = Src0 * (sq(u)*C1 + C2)*u, u = clip(Src1, +-C0):
    the h = sigma_o * tanh(c) stage in ONE pass, eliminating the separate
    tanh(c) instruction (ACT or DVE) and the tquad intermediate."""
    global _TANHMUL
    if _TANHMUL is not None:
        return _TANHMUL
    import concourse.dve_ops as dve_ops
    from concourse.dve_spec import (Src0, Src1, C0, C1, C2, Zero, minn, maxx,
                                    sq, Bin, AluOp, Spec, lower, _has_src1)
    from concourse.dve_uop import DveOpSpec

    name = "TANHMUL_ANT"
    for op in dve_ops.OPS:
        if op.name == name:
            _TANHMUL = op
            return op
    u = maxx(minn(Src1, C0), Bin(AluOp.SUBTRACT, Zero, C0))
    body = ((sq(u) * C1 + C2) * u) * Src0

    def ref(in0, in1, s0, s1, imm2):
        uu = np.clip(in1.astype(np.float32), -s0, s0)
        return in0 * ((uu * uu * s1 + imm2) * uu)

    spec = Spec(body=body, reference=ref)
    row = max(dve_ops._SUB_OPCODE_FOR_NAME.values()) + 1
    dve_ops._SUB_OPCODE_FOR_NAME[name] = row
    s = DveOpSpec(name=name, opcode=row, uops=lower(spec, ver="v3"),
                  rd1_en=_has_src1(spec))
    op = dve_ops.DveOp(name, spec, subdim=False, uops_sha={"v3": s.sha("v3")})
    dve_ops.OPS.append(op)
    dve_ops.CUSTOM_DVE_SPECS[name] = spec
    _TANHMUL = op
    return op


_TANH3 = None


def _get_tanh3():
    """Register the custom DVE op out = (sq(u)*C1 + C2)*u, u=clip(Src0,+-C0)
    via the documented dve_ops extension point (per-NEFF uop table)."""
    global _TANH3
    if _TANH3 is not None:
        return _TANH3
    import concourse.dve_ops as dve_ops
    from concourse.dve_spec import (Src0, C0, C1, C2, Zero, minn, maxx, sq,
                                    Bin, AluOp, Spec, lower, _has_src1)
    from concourse.dve_uop import DveOpSpec

    name = "TANH3_ANT"
    for op in dve_ops.OPS:
        if op.name == name:
            _TANH3 = op
            return op
    u = maxx(minn(Src0, C0), Bin(AluOp.SUBTRACT, Zero, C0))
    body = (sq(u) * C1 + C2) * u

    def ref(in0, in1, s0, s1, imm2):
        uu = np.clip(in0.astype(np.float32), -s0, s0)
        return (uu * uu * s1 + imm2) * uu

    spec = Spec(body=body, reference=ref)
    row = max(dve_ops._SUB_OPCODE_FOR_NAME.values()) + 1
    dve_ops._SUB_OPCODE_FOR_NAME[name] = row
    s = DveOpSpec(name=name, opcode=row, uops=lower(spec, ver="v3"),
                  rd1_en=_has_src1(spec))
    op = dve_ops.DveOp(name, spec, subdim=False, uops_sha={"v3": s.sha("v3")})
    dve_ops.OPS.append(op)
    dve_ops.CUSTOM_DVE_SPECS[name] = spec
    _TANH3 = op
    return op


def make_weights(w_ih, w_hh, b_ih, b_hh):
    """Block-diagonal weight mats [128, 320] fp16 for the pair matmuls.

    Row r = eta*64 + tap, tap = j*5 + c -- J-MAJOR, so the four halo tap
    rows j in {0,1} / {10,11} are contiguous 10-element runs (tap 60 =
    bias row in wx).  Col = eta*160 + G*40 + lam*5 + ch (lam-major, so the
    h-scatter destination taps j*5+c, j=2..9 are one contiguous 40-run in
    the same iteration order).  G in (i,f,o,g) order; rows of half eta
    only feed cols of half eta. g-block scaled 2x for the
    tanh-via-sigmoid trick.
    """
    refbase = (0, 5, 15, 10)  # i, f, o, g -> reference channel offsets
    w_ih = np.asarray(w_ih, np.float32)
    w_hh = np.asarray(w_hh, np.float32)
    bias = (np.asarray(b_ih) + np.asarray(b_hh)).astype(np.float32)
    wx = np.zeros((128, 320), np.float32)
    wh = np.zeros((128, 320), np.float32)
    for eta in range(2):
        r0, c0 = eta * 64, eta * 160
        for G in range(4):
            scale = 2.0 if G == 3 else 1.0
            for ch in range(C):
                for lam in range(NW):
                    col = c0 + G * 40 + lam * C + ch
                    for c in range(C):
                        for j in range(WJ):
                            k = j - lam
                            if 0 <= k < 5:
                                wx[r0 + j * C + c, col] = (
                                    scale * w_ih[refbase[G] + ch, c, k])
                                wh[r0 + j * C + c, col] = (
                                    scale * w_hh[refbase[G] + ch, c, k])
                    wx[r0 + BIAS_TAP, col] = scale * bias[refbase[G] + ch]
    return wx.astype(np.float16), wh.astype(np.float16)


def window_x_pairs(x):
    """[T, B, 5, 64] fp32 -> [T, B//256, 128, 8, 128] fp16 beta-block im2col.

    out[t, pair, beta*64+tap, w, b] = xpad[t, pair*256+beta*128+b,
    c, w*8 + j - 2] for tap = j*5+c < 60 (j-major); tap 60 = 1.0 (bias).
    Block w holds window w's taps for BOTH bgs of the pair (beta on the
    partition halves).
    """
    from numpy.lib.stride_tricks import sliding_window_view
    T, B = x.shape[0], x.shape[1]
    xpad = np.pad(x, ((0, 0), (0, 0), (0, 0), (2, 2)))
    win = sliding_window_view(xpad, WJ, axis=3)[:, :, :, ::8, :]  # T,B,C,8,12
    win = win.reshape(T, B // 256, 2, 128, C, NW, WJ)
    # [t, pair, beta, b, c, w, j] -> [t, pair, beta, j, c, w, b]
    arr = win.transpose(0, 1, 2, 6, 4, 5, 3)
    out = np.zeros((T, B // 256, 2, 64, NW, 128), np.float16)
    out[:, :, :, :60] = arr.reshape(T, B // 256, 2, 60, NW, 128)
    out[:, :, :, BIAS_TAP] = 1.0
    return out.reshape(T, B // 256, 128, NW, 128)


def _ap(base, off, dims):
    """Manual AP over the same tensor as `base` (an AP), keeping its
    partition dim, with free dims `dims` at extra element offset `off`."""
    return bass.AP(
        tensor=base.tensor,
        offset=base.offset + off,
        ap=[list(base.ap[0])] + [list(d) for d in dims],
    )


def build_body(tc, out_dram, xs, wx_d, wh_d, fcw5_d, consts_d, T, npair):
    nc = tc.nc
    f16, f32 = dt.float16, dt.float32
    tanh3 = _get_tanh3()
    tanhmul = _get_tanhmul()

    from contextlib import ExitStack
    es = ExitStack()
    pers = es.enter_context(tc.tile_pool(name="pers", bufs=1))
    psum_pool = es.enter_context(tc.tile_pool(name="psum", bufs=2, space="PSUM"))
    ifog_pool = es.enter_context(tc.tile_pool(name="ifog", bufs=6))
    small = es.enter_context(tc.tile_pool(name="small", bufs=7))
    xp_pool = es.enter_context(tc.tile_pool(name="xp", bufs=3))
    ht_pool = es.enter_context(tc.tile_pool(name="ht", bufs=6))
    fin_pool = es.enter_context(tc.tile_pool(name="fin", bufs=3))

    wx = pers.tile([128, 320], f16, tag="wx")
    nc.scalar.dma_start(out=wx, in_=wx_d)
    wh = pers.tile([128, 320], f16, tag="wh")
    nc.scalar.dma_start(out=wh, in_=wh_d)
    fcw5 = pers.tile([128, C * L], f16, tag="fcw5")
    nc.scalar.dma_start(
        out=fcw5,
        in_=bass.AP(tensor=fcw5_d.tensor, offset=fcw5_d.offset,
                    ap=[[0, 128], [1, C * L]]),
    )
    consts = pers.tile([128, 2], f32, tag="consts")
    nc.scalar.dma_start(
        out=consts,
        in_=bass.AP(tensor=consts_d.tensor, offset=consts_d.offset,
                    ap=[[0, 128], [1, 2]]),
    )
    fcbneg = consts[:, 0:1]
    negq = consts[:, 1:2]

    # h im2col buffers (64-tap pitch), one [128, 1024] per bg-pair, ping-pong
    xh = [[pers.tile([128, 1024], f16, tag=f"xh{pr}_{pp}", name=f"xh{pr}_{pp}")
           for pp in range(2)] for pr in range(npair)]
    for pr in range(npair):
        for pp in range(2):
            nc.gpsimd.memset(xh[pr][pp], 0.0)

    nquad = npair // 2
    cbuf = [[pers.tile([128, 1280], f16, tag=f"c{pp}_{q}", name=f"c{pp}_{q}")
             for q in range(nquad)] for pp in range(2)]
    for q in range(nquad):
        nc.vector.memset(cbuf[0][q], 0.0)
    tquad = [pers.tile([128, 1280], f16, tag=f"t{q}", name=f"t{q}")
             for q in range(nquad)]

    # one x tile per timestep: [128, npair*8*128].  t=0 is split per pair so
    # pair 0's matmuls start after 256KB, not after the whole 2MB.
    xt_all = xp_pool.tile([128, npair * 1024], f16, tag="xp")
    for pr in range(npair):
        nc.sync.dma_start(out=xt_all[:, pr * 1024 : (pr + 1) * 1024],
                          in_=xs[0, :, pr * 1024 : (pr + 1) * 1024])

    def emit_fc(pr):
        """Final FC + combine for both bgs of pair pr (reads xh[pr][T%2])."""
        nraw = fin_pool.tile([128, 2 * C], f32, tag="nraw")
        for beta in range(2):
            hview = _ap(xh[pr][T % 2][:], beta * 64 + 2 * C,
                        [[128, NW], [C, 8], [1, C]])
            fview = _ap(fcw5[:], 0, [[8 * C, NW], [C, 8], [1, C]])
            tmp5 = fin_pool.tile([128, C * L], f16, tag="tmp5")
            tview = _ap(tmp5[:], 0, [[8 * C, NW], [C, 8], [1, C]])
            nc.vector.tensor_tensor(out=tview, in0=hview, in1=fview,
                                    op=ALU.mult)
            nc.vector.tensor_reduce(
                out=nraw[:, beta * C : (beta + 1) * C],
                in_=tmp5[:].rearrange("p (l c) -> p c l", c=C),
                axis=mybir.AxisListType.X,
                op=ALU.add,
            )
        # sigmoid + channel-product + combine happen on HOST (postproc):
        # only the raw negated logits leave the device, shortening the tail.
        nc.sync.dma_start(out=out_dram[pr], in_=nraw[:])

    o_slices = {}
    vfq = {}
    for t in range(T):
        c_old, c_new = cbuf[t % 2], cbuf[(t + 1) % 2]
        # issue next timestep's x load FIRST (no deps -> drains immediately,
        # keeps the SP HWDGE ring free of head-of-line blocking)
        if t + 1 < T:
            xt_next = xp_pool.tile([128, npair * 1024], f16, tag="xp")
            nc.sync.dma_start(out=xt_next[:], in_=xs[t + 1])

        def emit_quad(q):
            """h-scatter + halos (+ final FC) for quad q, step t.  The
            tanh(c) is FUSED into the h-scatter custom op (TANHMUL_ANT:
            h = sigma_o * tanh3(cn) in one DVE pass), removing the
            separate tanh instruction and the tquad intermediate from
            the cn -> h -> transpose chain entirely."""
            for p2 in (2 * q, 2 * q + 1):
                h2 = p2 % 2
                xh2 = xh[p2][(t + 1) % 2][:]
                cnh = c_new[q][:, h2 * 640 : (h2 + 1) * 640]
                # j-major taps: dst j=2..9 is one contiguous 40-run
                hdst = _ap(xh2, 2 * C, [[64, 16], [1, 40]])
                nc.vector._custom_dve(tanhmul, out=hdst,
                                      in0=o_slices[p2], in1=cnh,
                                      s0=TL, s1=TB, imm2=TA)
                if t + 1 == T:
                    emit_fc(p2)
                if t + 1 < T:
                    # beta-merged halos; split across engines so both
                    # run in parallel (scatter->transpose chain)
                    # halo: j 10,11 of w<=6 (both betas) <- j 2,3 of w+1
                    # (copy from xh after the h-scatter, on gpsimd)
                    nc.gpsimd.tensor_copy(
                        out=_ap(xh2, 10 * C, [[64, 14], [1, 2 * C]]),
                        in_=_ap(xh2, 128 + 2 * C, [[64, 14], [1, 2 * C]]),
                    )
                    # halo: j 0,1 of w>=1 <- j 8,9 of w-1.  Quad 0's two
                    # copies run on ACT (cheaper per-op there, and DVE is
                    # the bottleneck); the rest stay on DVE to bound the
                    # ACT wait-queue parking risk.
                    if q == 0:
                        nc.scalar.copy(
                            out=_ap(xh2, 128 + 0, [[64, 14], [1, 2 * C]]),
                            in_=_ap(xh2, 8 * C, [[64, 14], [1, 2 * C]]),
                        )
                    else:
                        nc.vector.tensor_copy(
                            out=_ap(xh2, 128 + 0, [[64, 14], [1, 2 * C]]),
                            in_=_ap(xh2, 8 * C, [[64, 14], [1, 2 * C]]),
                        )

        for pr in range(npair):
            q, hp = pr // 2, pr % 2
            if t > 0:
                ht = ht_pool.tile([128, 2 * NK, 128], f16, tag="ht")
                nc.sync.dma_start(out=ht[:], in_=xh[pr][t % 2][:],
                                  transpose=True)
            ifog = ifog_pool.tile([128, 2 * NK * 320], f16, tag="ifog")
            for half in range(2):
                slot = psum_pool.tile([128, 2048], f32, tag="gates")
                for kw in range(NK):
                    w = half * NK + kw
                    out_mm = slot[:, kw * 512 : kw * 512 + 320]
                    xcol = (pr * 8 + w) * 128
                    nc.tensor.matmul(out_mm,
                                     lhsT=xt_all[:, xcol : xcol + 128],
                                     rhs=wx[:], start=True, stop=(t == 0))
                    if t > 0:
                        nc.tensor.matmul(out_mm, lhsT=ht[:, w, :],
                                         rhs=wh[:], start=False, stop=True)
                # sigmoid over 4 windows x both bgs;
                # ifog col = w*320 + beta*160 + G*40 + ch*8 + lam
                nc.scalar.activation(
                    out=ifog[:, half * 1280 : (half + 1) * 1280],
                    in_=_ap(slot[:], 0, [[512, NK], [1, 320]]),
                    func=ACT.Sigmoid,
                )

            # pair-wide cell update: (beta, w) merge into one 16-count dim
            ifog_f = ifog[:]
            sl_i = _ap(ifog_f, 0, [[160, 16], [1, 40]])
            sl_f = _ap(ifog_f, 40, [[160, 16], [1, 40]])
            sl_g = _ap(ifog_f, 120, [[160, 16], [1, 40]])
            o_slices[pr] = _ap(ifog_f, 80, [[160, 16], [1, 40]])

            # v = i * tanh(g) = (sigmoid(2g)*2 - 1) * i in ONE custom DVE op
            # (AFFINE_MUL_REDUCE: out = (in0*s0 + s1)*in1; exact math).
            # v and fc are QUAD-sized tiles written in pair halves, so the
            # cn add is one 1280-elem TT per quad (fewer DVE dispatches).
            if hp == 0:
                vq = small.tile([128, 1280], f16, tag="v", name="vq")
                fcq = small.tile([128, 1280], f16, tag="fc", name="fcq")
                vfq[q] = (vq, fcq)
            vq, fcq = vfq[q]
            vacc = fin_pool.tile([128, 1], f32, tag="vacc")
            nc.vector.affine_mul_reduce(
                out=vq[:, hp * 640 : (hp + 1) * 640], accum_out=vacc[:],
                in0=sl_g, in1=sl_i, scale=2.0, bias=-1.0)
            co = c_old[q][:, hp * 640 : (hp + 1) * 640]
            nc.vector.tensor_tensor(out=fcq[:, hp * 640 : (hp + 1) * 640],
                                    in0=sl_f, in1=co, op=ALU.mult)
            if hp == 1:
                nc.vector.tensor_tensor(out=c_new[q][:], in0=fcq[:],
                                        in1=vq[:], op=ALU.add)

            # quad blocks are emitted DEFERRED by one pair (see emit_quad
            # call sites below): the quad tanh's input cn(2q+1) is then
            # always ready when ACT reaches it, so it never parks in the
            # 4-deep wait queue blocking ready sigmoids behind it (this
            # parking was a measured ~1.8us ACT stall per timestep).
            if pr >= 2 and hp == 0:
                emit_quad(pr // 2 - 1)
        emit_quad(nquad - 1)
        if t + 1 < T:
            xt_all = xt_next

    es.close()


def host_prep(w_ih, w_hh, b_ih, b_hh, fc_w, fc_b, baseline):
    wx, wh = make_weights(w_ih, w_hh, b_ih, b_hh)
    fcw = np.asarray(fc_w)[0].astype(np.float32)           # (64,)
    # (l, c) layout to match the j-major h taps: col l*C + c = -fcw[l]
    fcw5 = np.repeat(-fcw, C)[None, :].astype(np.float16)  # (1, 320)
    base = float(np.asarray(baseline)[0])
    sig_base = 1.0 / (1.0 + np.exp(-base))
    consts = np.array([[-float(np.asarray(fc_b)[0]), -(1.0 - sig_base)]],
                      np.float32)
    return wx, wh, fcw5, consts


def build_program(T, npair):
    nc = bacc.Bacc("TRN2", target_bir_lowering=False, debug=False,
                   num_devices=1)
    xs = nc.dram_tensor("xs", [T, 128, npair * 2 * NK * 128], dt.float16,
                        kind="ExternalInput").ap()
    wx_d = nc.dram_tensor("wx", [128, 320], dt.float16,
                          kind="ExternalInput").ap()
    wh_d = nc.dram_tensor("wh", [128, 320], dt.float16,
                          kind="ExternalInput").ap()
    fcw5_d = nc.dram_tensor("fcw5", [1, C * L], dt.float16,
                            kind="ExternalInput").ap()
    consts_d = nc.dram_tensor("consts", [1, 2], dt.float32,
                              kind="ExternalInput").ap()
    out_d = nc.dram_tensor("out", [npair, 128, 2 * C], dt.float32,
                           kind="ExternalOutput").ap()
    with tile.TileContext(nc) as tc:
        build_body(tc, out_d, xs, wx_d, wh_d, fcw5_d, consts_d, T, npair)
    nc.compile()
    return nc


_PROG_CACHE = {}


def prepare(x, w_ih, w_hh, b_ih, b_hh, fc_w, fc_b, baseline):
    x = np.asarray(x)
    T, B = x.shape[0], x.shape[1]
    npair = (B // NCORES) // 256
    key = (T, npair)
    if key not in _PROG_CACHE:
        _PROG_CACHE[key] = build_program(T, npair)
    nc = _PROG_CACHE[key]

    wx, wh, fcw5, consts = host_prep(w_ih, w_hh, b_ih, b_hh, fc_w, fc_b,
                                     baseline)
    xw = window_x_pairs(x)          # [T, pairs_glob, 128, 8, 128]
    in_maps = []
    for core in range(NCORES):
        xc = xw[:, core * npair : (core + 1) * npair]
        # [T, npair, 128, 8, 128] -> [T, 128, npair*8*128]
        xc = xc.transpose(0, 2, 1, 3, 4).reshape(
            xw.shape[0], 128, npair * 2 * NK * 128)
        in_maps.append({
            "xs": np.ascontiguousarray(xc),
            "wx": wx,
            "wh": wh,
            "fcw5": fcw5,
            "consts": consts,
        })

    fcb = float(np.asarray(fc_b)[0])
    sig_base = 1.0 / (1.0 + np.exp(-float(np.asarray(baseline)[0])))

    def postproc(res):
        # device ships negated raw logits; sigmoid/product/combine are host
        outs = []
        for r in res.results:
            nraw = r["out"].astype(np.float32)          # [npair, 128, 2C]
            pbar = 1.0 / (1.0 + np.exp(-(nraw - fcb)))  # = 1 - p
            prod = pbar.reshape(npair, 128, 2, C).prod(axis=3)
            resv = 1.0 - (1.0 - sig_base) * prod        # [npair, 128, 2]
            outs.append(resv.transpose(0, 2, 1).reshape(-1))
        return np.concatenate(outs).astype(np.float32)

    return nc, in_maps, postproc


def kernel(x, w_ih, w_hh, b_ih, b_hh, fc_w, fc_b, baseline):
    nc, in_maps, postproc = prepare(x, w_ih, w_hh, b_ih, b_hh, fc_w, fc_b,
                                    baseline)
    res = bass_utils.run_bass_kernel_spmd(nc, in_maps,
                                          core_ids=list(range(NCORES)))
    return postproc(res)



# revision 70
# speedup vs baseline: 1.0147x; 1.0147x over previous
"""ConvLSTM classifier kernel for Trainium2 (8 NeuronCores, data-parallel). v2

Math (per core, batch shard BL=2048):
  for t in 0..T-1:
    gates = conv1d(x_t, w_ih) + conv1d(h, w_hh) + bias     # (BL, 20, 64), 'SAME' K=5
    i,f,g,o = split(gates); i,f,o = sigmoid; g = tanh
    c = f*c + i*g ; h = o*tanh(c)
  logit = h . fc_w + fc_b ; p = sigmoid(logit)
  out = 1 - prod_c(1-p_c) * (1-sigmoid(baseline))

Design (v2.5, ~426-449us/core measured, run-to-run thermal variance ~3%;
ACT engine ~85% active = the algorithm's sigmoid/tanh stream is the wall):
  - J-MAJOR tap order (tap = j*5+c) and lam-major gate columns
    (col = G*40 + lam*5 + ch): halo copies are packed 10-element runs and
    the h-scatter is one contiguous 40-run per window block (the (c,j)
    order forced 2-element bursts that cost ~520ns per halo copy).
  - v = i*tanh(g) = (sigmoid(2g)*2-1)*i in ONE AFFINE_MUL_REDUCE custom
    DVE op (exact; replaces the tensor_scalar + tensor_tensor pair).
  - tanh(c) is FUSED into the h-scatter: h = sigma_o * tanh3(c) in ONE
    custom DVE uop (TANHMUL_ANT, 7 ALU blocks: clamp+cubic+multiply,
    strided sigma_o as in0, contiguous cn as the required 1-D in1).
    No separate tanh instruction, no tquad intermediate, shorter
    cn -> h -> transpose chain.  End-to-end rel err 1.2e-3.
  - Quad blocks (tanh + h-scatter + halos) are emitted deferred by one
    pair: the quad tanh's cn input is then always ready when the engine
    reaches it, so it never parks in the 4-deep wait queue blocking
    ready sigmoids behind it (measured ~1.8us/step ACT stall).
  - Halos: left is a DVE copy, right a gpsimd copy (parallel engines).
    A gpsimd op must NOT read the ifog pool tile (its WAR release then
    makes later ACT sigmoids wait on the slow gpsimd queue; +98us).
  - v and fc live in QUAD-sized tiles written in pair halves, so the
    cn = fc + v add is one 1280-elem TT per quad (halves the dispatch
    count of the DVE bottleneck's cheapest-per-element op).
  - Final sigmoid/product/combine on HOST; device ships raw logits.
  - Rejected by measurement: fused x|h matmuls + per-gate ACT slices
    (794us: chip power-throttle caps utilization when 5 engines run hot,
    and small strided-PSUM ACT instructions carry ~260ns fixed cost);
    gpsimd h-multiply (throttle); 2-step x prefetch and split x DMA
    (DMA contention with the transposes); halos both on gpsimd (the
    h->halo->transpose chain gates the next step's matmuls, +17us);
    halos recomputed as gpsimd o*tanh TTs (+98us: they read the ifog
    pool tile inside the deferred block, so the pool's WAR dep makes
    later ACT sigmoids wait on the slow gpsimd queue); ifog bufs 6->8
    (neutral-to-worse); halo-left as a small SBUF->SBUF DMA on the sync
    queue (+290us!! tiny DMAs wreck the HWDGE ring that also carries
    the x stream and h transposes, despite ample chain slack).

Original v2 design notes:
  - x im2col is pre-transposed ON HOST into pair-block layout; one
    contiguous [128, 8192] HWDGE DMA per timestep (issued at the top of
    each t so the SP ring never head-of-line blocks; t=0 split per pair
    so pair 0's matmuls start early).
  - Only h goes through the on-device DMA-xbar transpose, at 64-tap
    pitch: [128 b, 1024] -> [128, 8 blk, 128 b] per bg-PAIR.
  - Beta-interleaved pair blocks: stationary tile block w holds window
    w's taps for BOTH bgs of a pair (bg0 taps at partitions 0-63, bg1
    at 64-127); rhs weights are block-diagonal [64x160 | 64x160] so one
    N=320 matmul yields window w's gates for both bgs.  Per pair: 8
    window-MM pairs (x start / h stop, PSUM-accumulated), 4-bank PSUM
    slot per half, 2 slots rotating.
  - Bias enters via a constant-1.0 row (tap 60) in the HOST x data;
    g-gate weights doubled so a single Sigmoid pass covers all four
    gates (tanh(g) = 2*sigmoid(2g)-1).
  - ACT (the bottleneck engine, ~84% busy): one 1280-elem Sigmoid per
    4-window half + one 1280-elem Tanh per QUAD (4 bgs) + final FC.
  - DVE cell update at bg-PAIR granularity (640-elem ops): v=i*g,
    u=2v-i (STT), fc=f*c, cn=fc+u, h-scatter TT; the two beta-merged
    halo copies split GpSimd/DVE so they run in parallel.
  - Final FC emitted inside the t=15 scatter block per pair (pair-wide
    sigmoid/product), overlapping the remaining pairs' last-step work.
"""

import numpy as np

import concourse.bass as bass
import concourse.bacc as bacc
import concourse.tile as tile
import concourse.mybir as mybir
from concourse import bass_utils

dt = mybir.dt
ALU = mybir.AluOpType
ACT = mybir.ActivationFunctionType

TIME = 16
BATCH = 16384
C = 5
L = 64
NCORES = 8
BL = BATCH // NCORES          # 2048 per core
NBG = BL // 128               # 16 batch groups
NPAIR = NBG // 2              # 8 bg pairs
NW = 8                        # l-windows per batch row (l_seg = 8)
WJ = 12                       # taps per (window, channel): 8 + 4 halo
NK = 4                        # window-pairs per bg
BIAS_TAP = 60                 # constant-1.0 row inside each 64-tap half

# clamped-cubic tanh fit for tanh(c): t ~ u*(TA + TB*u^2), u = clip(c,-TL,TL)
# (|c| measured <= 1.53; end-to-end rel err vs oracle 1.2e-3, tol 2e-2)
TL = 2.0
TA = 0.852982
TB = -0.097015

_TANHMUL = None


def _get_tanhmul():
    """Custom DVE op out = Src0 * (sq(u)*C1 + C2)*u, u = clip(Src1, +-C0):
    the h = sigma_o * tanh(c) stage in ONE pass, eliminating the separate
    tanh(c) instruction (ACT or DVE) and the tquad intermediate."""
    global _TANHMUL
    if _TANHMUL is not None:
        return _TANHMUL
    import concourse.dve_ops as dve_ops
    from concourse.dve_spec import (Src0, Src1, C0, C1, C2, Zero, minn, maxx,
                                    sq, Bin, AluOp, Spec, lower, _has_src1)
    from concourse.dve_uop import DveOpSpec

    name = "TANHMUL_ANT"
    for op in dve_ops.OPS:
        if op.name == name:
            _TANHMUL = op
            return op
    u = maxx(minn(Src1, C0), Bin(AluOp.SUBTRACT, Zero, C0))
    body = ((sq(u) * C1 + C2) * u) * Src0

    def ref(in0, in1, s0, s1, imm2):
        uu = np.clip(in1.astype(np.float32), -s0, s0)
        return in0 * ((uu * uu * s1 + imm2) * uu)

    spec = Spec(body=body, reference=ref)
    row = max(dve_ops._SUB_OPCODE_FOR_NAME.values()) + 1
    dve_ops._SUB_OPCODE_FOR_NAME[name] = row
    s = DveOpSpec(name=name, opcode=row, uops=lower(spec, ver="v3"),
                  rd1_en=_has_src1(spec))
    op = dve_ops.DveOp(name, spec, subdim=False, uops_sha={"v3": s.sha("v3")})
    dve_ops.OPS.append(op)
    dve_ops.CUSTOM_DVE_SPECS[name] = spec
    _TANHMUL = op
    return op


_TANH3 = None


def _get_tanh3():
    """Register the custom DVE op out = (sq(u)*C1 + C2)*u, u=clip(Src0,+-C0)
    via the documented dve_ops extension point (per-NEFF uop table)."""
    global _TANH3
    if _TANH3 is not None:
        return _TANH3
    import concourse.dve_ops as dve_ops
    from concourse.dve_spec import (Src0, C0, C1, C2, Zero, minn, maxx, sq,
                                    Bin, AluOp, Spec, lower, _has_src1)
    from concourse.dve_uop import DveOpSpec

    name = "TANH3_ANT"
    for op in dve_ops.OPS:
        if op.name == name:
            _TANH3 = op
            return op
    u = maxx(minn(Src0, C0), Bin(AluOp.SUBTRACT, Zero, C0))
    body = (sq(u) * C1 + C2) * u

    def ref(in0, in1, s0, s1, imm2):
        uu = np.clip(in0.astype(np.float32), -s0, s0)
        return (uu * uu * s1 + imm2) * uu

    spec = Spec(body=body, reference=ref)
    row = max(dve_ops._SUB_OPCODE_FOR_NAME.values()) + 1
    dve_ops._SUB_OPCODE_FOR_NAME[name] = row
    s = DveOpSpec(name=name, opcode=row, uops=lower(spec, ver="v3"),
                  rd1_en=_has_src1(spec))
    op = dve_ops.DveOp(name, spec, subdim=False, uops_sha={"v3": s.sha("v3")})
    dve_ops.OPS.append(op)
    dve_ops.CUSTOM_DVE_SPECS[name] = spec
    _TANH3 = op
    return op


def make_weights(w_ih, w_hh, b_ih, b_hh):
    """Block-diagonal weight mats [128, 320] fp16 for the pair matmuls.

    Row r = eta*64 + tap, tap = j*5 + c -- J-MAJOR, so the four halo tap
    rows j in {0,1} / {10,11} are contiguous 10-element runs (tap 60 =
    bias row in wx).  Col = eta*160 + G*40 + lam*5 + ch (lam-major, so the
    h-scatter destination taps j*5+c, j=2..9 are one contiguous 40-run in
    the same iteration order).  G in (i,f,o,g) order; rows of half eta
    only feed cols of half eta. g-block scaled 2x for the
    tanh-via-sigmoid trick.
    """
    refbase = (0, 5, 15, 10)  # i, f, o, g -> reference channel offsets
    w_ih = np.asarray(w_ih, np.float32)
    w_hh = np.asarray(w_hh, np.float32)
    bias = (np.asarray(b_ih) + np.asarray(b_hh)).astype(np.float32)
    wx = np.zeros((128, 320), np.float32)
    wh = np.zeros((128, 320), np.float32)
    for eta in range(2):
        r0, c0 = eta * 64, eta * 160
        for G in range(4):
            scale = 2.0 if G == 3 else 1.0
            for ch in range(C):
                for lam in range(NW):
                    col = c0 + G * 40 + lam * C + ch
                    for c in range(C):
                        for j in range(WJ):
                            k = j - lam
                            if 0 <= k < 5:
                                wx[r0 + j * C + c, col] = (
                                    scale * w_ih[refbase[G] + ch, c, k])
                                wh[r0 + j * C + c, col] = (
                                    scale * w_hh[refbase[G] + ch, c, k])
                    wx[r0 + BIAS_TAP, col] = scale * bias[refbase[G] + ch]
    return wx.astype(np.float16), wh.astype(np.float16)


def window_x_pairs(x):
    """[T, B, 5, 64] fp32 -> [T, B//256, 128, 8, 128] fp16 beta-block im2col.

    out[t, pair, beta*64+tap, w, b] = xpad[t, pair*256+beta*128+b,
    c, w*8 + j - 2] for tap = j*5+c < 60 (j-major); tap 60 = 1.0 (bias).
    Block w holds window w's taps for BOTH bgs of the pair (beta on the
    partition halves).
    """
    from numpy.lib.stride_tricks import sliding_window_view
    T, B = x.shape[0], x.shape[1]
    xpad = np.pad(x, ((0, 0), (0, 0), (0, 0), (2, 2)))
    win = sliding_window_view(xpad, WJ, axis=3)[:, :, :, ::8, :]  # T,B,C,8,12
    win = win.reshape(T, B // 256, 2, 128, C, NW, WJ)
    # [t, pair, beta, b, c, w, j] -> [t, pair, beta, j, c, w, b]
    arr = win.transpose(0, 1, 2, 6, 4, 5, 3)
    out = np.zeros((T, B // 256, 2, 64, NW, 128), np.float16)
    out[:, :, :, :60] = arr.reshape(T, B // 256, 2, 60, NW, 128)
    out[:, :, :, BIAS_TAP] = 1.0
    return out.reshape(T, B // 256, 128, NW, 128)


def _ap(base, off, dims):
    """Manual AP over the same tensor as `base` (an AP), keeping its
    partition dim, with free dims `dims` at extra element offset `off`."""
    return bass.AP(
        tensor=base.tensor,
        offset=base.offset + off,
        ap=[list(base.ap[0])] + [list(d) for d in dims],
    )


def build_body(tc, out_dram, xs, wx_d, wh_d, fcw5_d, consts_d, T, npair):
    nc = tc.nc
    f16, f32 = dt.float16, dt.float32
    tanh3 = _get_tanh3()
    tanhmul = _get_tanhmul()

    from contextlib import ExitStack
    es = ExitStack()
    pers = es.enter_context(tc.tile_pool(name="pers", bufs=1))
    psum_pool = es.enter_context(tc.tile_pool(name="psum", bufs=2, space="PSUM"))
    ifog_pool = es.enter_context(tc.tile_pool(name="ifog", bufs=6))
    small = es.enter_context(tc.tile_pool(name="small", bufs=7))
    xp_pool = es.enter_context(tc.tile_pool(name="xp", bufs=3))
    ht_pool = es.enter_context(tc.tile_pool(name="ht", bufs=6))
    fin_pool = es.enter_context(tc.tile_pool(name="fin", bufs=3))

    wx = pers.tile([128, 320], f16, tag="wx")
    nc.scalar.dma_start(out=wx, in_=wx_d)
    wh = pers.tile([128, 320], f16, tag="wh")
    nc.scalar.dma_start(out=wh, in_=wh_d)
    fcw5 = pers.tile([128, C * L], f16, tag="fcw5")
    nc.scalar.dma_start(
        out=fcw5,
        in_=bass.AP(tensor=fcw5_d.tensor, offset=fcw5_d.offset,
                    ap=[[0, 128], [1, C * L]]),
    )
    consts = pers.tile([128, 2], f32, tag="consts")
    nc.scalar.dma_start(
        out=consts,
        in_=bass.AP(tensor=consts_d.tensor, offset=consts_d.offset,
                    ap=[[0, 128], [1, 2]]),
    )
    fcbneg = consts[:, 0:1]
    negq = consts[:, 1:2]

    # h im2col buffers (64-tap pitch), one [128, 1024] per bg-pair, ping-pong
    xh = [[pers.tile([128, 1024], f16, tag=f"xh{pr}_{pp}", name=f"xh{pr}_{pp}")
           for pp in range(2)] for pr in range(npair)]
    for pr in range(npair):
        for pp in range(2):
            nc.gpsimd.memset(xh[pr][pp], 0.0)

    nquad = npair // 2
    cbuf = [[pers.tile([128, 1280], f16, tag=f"c{pp}_{q}", name=f"c{pp}_{q}")
             for q in range(nquad)] for pp in range(2)]
    for q in range(nquad):
        nc.vector.memset(cbuf[0][q], 0.0)
    tquad = [pers.tile([128, 1280], f16, tag=f"t{q}", name=f"t{q}")
             for q in range(nquad)]

    # one x tile per timestep: [128, npair*8*128].  t=0 is split per pair so
    # pair 0's matmuls start after 256KB, not after the whole 2MB.
    xt_all = xp_pool.tile([128, npair * 1024], f16, tag="xp")
    for pr in range(npair):
        nc.sync.dma_start(out=xt_all[:, pr * 1024 : (pr + 1) * 1024],
                          in_=xs[0, :, pr * 1024 : (pr + 1) * 1024])

    def emit_fc(pr):
        """Final FC + combine for both bgs of pair pr (reads xh[pr][T%2])."""
        nraw = fin_pool.tile([128, 2 * C], f32, tag="nraw")
        for beta in range(2):
            hview = _ap(xh[pr][T % 2][:], beta * 64 + 2 * C,
                        [[128, NW], [C, 8], [1, C]])
            fview = _ap(fcw5[:], 0, [[8 * C, NW], [C, 8], [1, C]])
            tmp5 = fin_pool.tile([128, C * L], f16, tag="tmp5")
            tview = _ap(tmp5[:], 0, [[8 * C, NW], [C, 8], [1, C]])
            nc.vector.tensor_tensor(out=tview, in0=hview, in1=fview,
                                    op=ALU.mult)
            nc.vector.tensor_reduce(
                out=nraw[:, beta * C : (beta + 1) * C],
                in_=tmp5[:].rearrange("p (l c) -> p c l", c=C),
                axis=mybir.AxisListType.X,
                op=ALU.add,
            )
        # sigmoid + channel-product + combine happen on HOST (postproc):
        # only the raw negated logits leave the device, shortening the tail.
        nc.sync.dma_start(out=out_dram[pr], in_=nraw[:])

    o_slices = {}
    vfq = {}
    for t in range(T):
        c_old, c_new = cbuf[t % 2], cbuf[(t + 1) % 2]
        # issue next timestep's x load FIRST (no deps -> drains immediately,
        # keeps the SP HWDGE ring free of head-of-line blocking)
        if t + 1 < T:
            xt_next = xp_pool.tile([128, npair * 1024], f16, tag="xp")
            nc.sync.dma_start(out=xt_next[:], in_=xs[t + 1])

        def emit_quad(q):
            """h-scatter + halos (+ final FC) for quad q, step t.  The
            tanh(c) is FUSED into the h-scatter custom op (TANHMUL_ANT:
            h = sigma_o * tanh3(cn) in one DVE pass), removing the
            separate tanh instruction and the tquad intermediate from
            the cn -> h -> transpose chain entirely."""
            for p2 in (2 * q, 2 * q + 1):
                h2 = p2 % 2
                xh2 = xh[p2][(t + 1) % 2][:]
                cnh = c_new[q][:, h2 * 640 : (h2 + 1) * 640]
                # j-major taps: dst j=2..9 is one contiguous 40-run
                hdst = _ap(xh2, 2 * C, [[64, 16], [1, 40]])
                nc.vector._custom_dve(tanhmul, out=hdst,
                                      in0=o_slices[p2], in1=cnh,
                                      s0=TL, s1=TB, imm2=TA)
                if t + 1 == T:
                    emit_fc(p2)
                if t + 1 < T:
                    # beta-merged halos; split across engines so both
                    # run in parallel (scatter->transpose chain)
                    # halo: j 10,11 of w<=6 (both betas) <- j 2,3 of w+1
                    # (copy from xh after the h-scatter, on gpsimd)
                    nc.gpsimd.tensor_copy(
                        out=_ap(xh2, 10 * C, [[64, 14], [1, 2 * C]]),
                        in_=_ap(xh2, 128 + 2 * C, [[64, 14], [1, 2 * C]]),
                    )
                    # halo: j 0,1 of w>=1 <- j 8,9 of w-1 (copy, DVE)
                    nc.vector.tensor_copy(
                        out=_ap(xh2, 128 + 0, [[64, 14], [1, 2 * C]]),
                        in_=_ap(xh2, 8 * C, [[64, 14], [1, 2 * C]]),
                    )

        for pr in range(npair):
            q, hp = pr // 2, pr % 2
            if t > 0:
                ht = ht_pool.tile([128, 2 * NK, 128], f16, tag="ht")
                nc.sync.dma_start(out=ht[:], in_=xh[pr][t % 2][:],
                                  transpose=True)
            ifog = ifog_pool.tile([128, 2 * NK * 320], f16, tag="ifog")
            for half in range(2):
                slot = psum_pool.tile([128, 2048], f32, tag="gates")
                for kw in range(NK):
                    w = half * NK + kw
                    out_mm = slot[:, kw * 512 : kw * 512 + 320]
                    xcol = (pr * 8 + w) * 128
                    nc.tensor.matmul(out_mm,
                                     lhsT=xt_all[:, xcol : xcol + 128],
                                     rhs=wx[:], start=True, stop=(t == 0))
                    if t > 0:
                        nc.tensor.matmul(out_mm, lhsT=ht[:, w, :],
                                         rhs=wh[:], start=False, stop=True)
                # sigmoid over 4 windows x both bgs;
                # ifog col = w*320 + beta*160 + G*40 + ch*8 + lam
                nc.scalar.activation(
                    out=ifog[:, half * 1280 : (half + 1) * 1280],
                    in_=_ap(slot[:], 0, [[512, NK], [1, 320]]),
                    func=ACT.Sigmoid,
                )

            # pair-wide cell update: (beta, w) merge into one 16-count dim
            ifog_f = ifog[:]
            sl_i = _ap(ifog_f, 0, [[160, 16], [1, 40]])
            sl_f = _ap(ifog_f, 40, [[160, 16], [1, 40]])
            sl_g = _ap(ifog_f, 120, [[160, 16], [1, 40]])
            o_slices[pr] = _ap(ifog_f, 80, [[160, 16], [1, 40]])

            # v = i * tanh(g) = (sigmoid(2g)*2 - 1) * i in ONE custom DVE op
            # (AFFINE_MUL_REDUCE: out = (in0*s0 + s1)*in1; exact math).
            # v and fc are QUAD-sized tiles written in pair halves, so the
            # cn add is one 1280-elem TT per quad (fewer DVE dispatches).
            if hp == 0:
                vq = small.tile([128, 1280], f16, tag="v", name="vq")
                fcq = small.tile([128, 1280], f16, tag="fc", name="fcq")
                vfq[q] = (vq, fcq)
            vq, fcq = vfq[q]
            vacc = fin_pool.tile([128, 1], f32, tag="vacc")
            nc.vector.affine_mul_reduce(
                out=vq[:, hp * 640 : (hp + 1) * 640], accum_out=vacc[:],
                in0=sl_g, in1=sl_i, scale=2.0, bias=-1.0)
            co = c_old[q][:, hp * 640 : (hp + 1) * 640]
            nc.vector.tensor_tensor(out=fcq[:, hp * 640 : (hp + 1) * 640],
                                    in0=sl_f, in1=co, op=ALU.mult)
            if hp == 1:
                nc.vector.tensor_tensor(out=c_new[q][:], in0=fcq[:],
                                        in1=vq[:], op=ALU.add)

            # quad blocks are emitted DEFERRED by one pair (see emit_quad
            # call sites below): the quad tanh's input cn(2q+1) is then
            # always ready when ACT reaches it, so it never parks in the
            # 4-deep wait queue blocking ready sigmoids behind it (this
            # parking was a measured ~1.8us ACT stall per timestep).
            if pr >= 2 and hp == 0:
                emit_quad(pr // 2 - 1)
        emit_quad(nquad - 1)
        if t + 1 < T:
            xt_all = xt_next

    es.close()


def host_prep(w_ih, w_hh, b_ih, b_hh, fc_w, fc_b, baseline):
    wx, wh = make_weights(w_ih, w_hh, b_ih, b_hh)
    fcw = np.asarray(fc_w)[0].astype(np.float32)           # (64,)
    # (l, c) layout to match the j-major h taps: col l*C + c = -fcw[l]
    fcw5 = np.repeat(-fcw, C)[None, :].astype(np.float16)  # (1, 320)
    base = float(np.asarray(baseline)[0])
    sig_base = 1.0 / (1.0 + np.exp(-base))
    consts = np.array([[-float(np.asarray(fc_b)[0]), -(1.0 - sig_base)]],
                      np.float32)
    return wx, wh, fcw5, consts


def build_program(T, npair):
    nc = bacc.Bacc("TRN2", target_bir_lowering=False, debug=False,
                   num_devices=1)
    xs = nc.dram_tensor("xs", [T, 128, npair * 2 * NK * 128], dt.float16,
                        kind="ExternalInput").ap()
    wx_d = nc.dram_tensor("wx", [128, 320], dt.float16,
                          kind="ExternalInput").ap()
    wh_d = nc.dram_tensor("wh", [128, 320], dt.float16,
                          kind="ExternalInput").ap()
    fcw5_d = nc.dram_tensor("fcw5", [1, C * L], dt.float16,
                            kind="ExternalInput").ap()
    consts_d = nc.dram_tensor("consts", [1, 2], dt.float32,
                              kind="ExternalInput").ap()
    out_d = nc.dram_tensor("out", [npair, 128, 2 * C], dt.float32,
                           kind="ExternalOutput").ap()
    with tile.TileContext(nc) as tc:
        build_body(tc, out_d, xs, wx_d, wh_d, fcw5_d, consts_d, T, npair)
    nc.compile()
    return nc


_PROG_CACHE = {}


def prepare(x, w_ih, w_hh, b_ih, b_hh, fc_w, fc_b, baseline):
    x = np.asarray(x)
    T, B = x.shape[0], x.shape[1]
    npair = (B // NCORES) // 256
    key = (T, npair)
    if key not in _PROG_CACHE:
        _PROG_CACHE[key] = build_program(T, npair)
    nc = _PROG_CACHE[key]

    wx, wh, fcw5, consts = host_prep(w_ih, w_hh, b_ih, b_hh, fc_w, fc_b,
                                     baseline)
    xw = window_x_pairs(x)          # [T, pairs_glob, 128, 8, 128]
    in_maps = []
    for core in range(NCORES):
        xc = xw[:, core * npair : (core + 1) * npair]
        # [T, npair, 128, 8, 128] -> [T, 128, npair*8*128]
        xc = xc.transpose(0, 2, 1, 3, 4).reshape(
            xw.shape[0], 128, npair * 2 * NK * 128)
        in_maps.append({
            "xs": np.ascontiguousarray(xc),
            "wx": wx,
            "wh": wh,
            "fcw5": fcw5,
            "consts": consts,
        })

    fcb = float(np.asarray(fc_b)[0])
    sig_base = 1.0 / (1.0 + np.exp(-float(np.asarray(baseline)[0])))

    def postproc(res):
        # device ships negated raw logits; sigmoid/product/combine are host
        outs = []
        for r in res.results:
            nraw = r["out"].astype(np.float32)          # [npair, 128, 2C]
            pbar = 1.0 / (1.0 + np.exp(-(nraw - fcb)))  # = 1 - p
            prod = pbar.reshape(npair, 128, 2, C).prod(axis=3)
            resv = 1.0 - (1.0 - sig_base) * prod        # [npair, 128, 2]
            outs.append(resv.transpose(0, 2, 1).reshape(-1))
        return np.concatenate(outs).astype(np.float32)

    return nc, in_maps, postproc


def kernel(x, w_ih, w_hh, b_ih, b_hh, fc_w, fc_b, baseline):
    nc, in_maps, postproc = prepare(x, w_ih, w_hh, b_ih, b_hh, fc_w, fc_b,
                                    baseline)
    res = bass_utils.run_bass_kernel_spmd(nc, in_maps,
                                          core_ids=list(range(NCORES)))
    return postproc(res)

